# revision 25
# baseline (speedup 1.0000x reference)
"""Batched greedy GRU decoder on 8 Trainium2 NeuronCores.

Strategy: tensor-parallel over the vocabulary. W_proj [32000,512] fp32 (65.5MB)
cannot fit in one core's 28MB SBUF, but an 8-way shard (padded to 4096
rows/core) stays SBUF-resident across all 64 decode steps.

Optimizations vs the 2.56ms baseline (measured 2.52ms):
  - projection runs as an exact fp16 hi/lo 3-pass decomposition
    (hi@hi + hi@lo + lo@hi; the dropped lo@lo term is ~2^-24*|h||w|) at
    1 cycle/row instead of fp32's 4 cycles/row. (fp32r was tested on HW:
    1 cy/row but only ~1e-4 accurate and rejects col-tiling - unusable
    for exact argmax. The fp16 3-pass win is partially eaten by per-group
    LDWEIGHTS serialization against in-flight same-col-group matmuls.)
  - projection split in two [128,512] PSUM halves; half 0's
    MAX8/FIND_INDEX8 runs under half 1's matmuls, then a short
    strictly-greater merge (ties keep the lower vocab index).
  - PE HAM clock-gate warm-keeping: next-step gate matmuls + projection
    bias init + dummy matmuls fill the AllGather wait, and tiny "heartbeat"
    matmuls (dependency tensor as the stationary operand, 32-col moving)
    fire down the token-dependent chain (tok, gx, rp, zt, tmp, n) so the
    PE is never idle >3.4us during the serial GRU phase.
  - the whole GRU serial phase runs in a [64, 256] layout (partition
    32q+b holds h-range 256q..256q+255): gate matmuls 2-way col-tiled,
    gx gathered by two per-half indirect DMAs from host-preshuffled
    tables, every DVE/ACT op in the token-critical chain at 64 lanes
    instead of 32 (~3.75us vs 5.7us); transposes read the two
    partition-halves via a stacked identity.
  - candidate DMA on the scalar queue; AllGather readback split into two
    half-gathers on scalar+sync queues.
Measured: 2392067 ns, 0/2048 token mismatches (baseline 2563612).
"""
import numpy as np

V, E, H, B = 32000, 256, 512, 32
PAD, EOS, SOS = 0, 1, 2
N_CORES = 8
VS = 4096           # padded vocab entries per core
VPAD = VS * N_CORES  # 32768
NEG = -1.0e30
N_DUMMY = 5


def _build(T: int):
    import concourse.bass as bass
    import concourse.bacc as bacc
    import concourse.mybir as mybir
    from concourse.tile import TileContext

    F32 = mybir.dt.float32
    F16 = mybir.dt.float16
    U32 = mybir.dt.uint32
    I32 = mybir.dt.int32
    AF = mybir.ActivationFunctionType
    OP = mybir.AluOpType

    nc = bacc.Bacc(None)

    wph_in = nc.declare_dram_parameter("wph", [H, VS], F16, isOutput=False)
    wpl_in = nc.declare_dram_parameter("wpl", [H, VS], F16, isOutput=False)
    bproj_in = nc.declare_dram_parameter("bproj", [1, VS], F32, isOutput=False)
    whh_in = nc.declare_dram_parameter("whhT", [H, 3 * H], F32, isOutput=False)
    bnh_in = nc.declare_dram_parameter("b_nh", [1, H], F32, isOutput=False)
    gtabq_in = [nc.declare_dram_parameter(f"gtabq{q}", [V, 3 * 256], F32,
                                          isOutput=False) for q in range(2)]
    h0_in = nc.declare_dram_parameter("h0", [2 * B, 256], F32, isOutput=False)
    h0T_in = nc.declare_dram_parameter("h0T", [H, B], F32, isOutput=False)
    ident_in = nc.declare_dram_parameter("ident", [B, B], F32, isOutput=False)
    ident64_in = nc.declare_dram_parameter("ident64", [2 * B, B], F32, isOutput=False)
    ones_in = nc.declare_dram_parameter("ones", [1, B], F32, isOutput=False)
    pbase_in = nc.declare_dram_parameter("pbase", [128, 1], F32, isOutput=False)

    toks_out = nc.declare_dram_parameter("toks", [B, T], I32, isOutput=True)

    cc_ins = [nc.dram_tensor(f"cc_in_{t}", [128, 2], F32) for t in range(T)]
    cc_outs = [
        nc.dram_tensor(f"cc_out_{t}", [N_CORES * 128, 2], F32, addr_space="Shared")
        for t in range(T)
    ]

    with TileContext(nc) as tc:
        with (
            tc.tile_pool(name="wpool", bufs=1) as wpool,
            tc.tile_pool(name="state", bufs=1) as state,
            tc.tile_pool(name="sb", bufs=2) as sb,
            tc.tile_pool(name="ps_gate", bufs=1, space="PSUM") as ps_gate,
            tc.tile_pool(name="ps_tp", bufs=1, space="PSUM") as ps_tp,
            tc.tile_pool(name="ps_proj", bufs=1, space="PSUM") as ps_proj,
            tc.tile_pool(name="ps_dummy", bufs=1, space="PSUM") as ps_dummy,
        ):
            # ---------- SBUF-resident weights ----------
            wph_sb, wpl_sb = [], []
            for k in range(4):
                w = wpool.tile([128, VS], F16, tag=f"wph{k}")
                nc.sync.dma_start(out=w[:], in_=wph_in[128 * k:128 * (k + 1), :])
                wph_sb.append(w)
            for k in range(4):
                w = wpool.tile([128, VS], F16, tag=f"wpl{k}")
                nc.sync.dma_start(out=w[:], in_=wpl_in[128 * k:128 * (k + 1), :])
                wpl_sb.append(w)
            whh_sb = []
            for k in range(4):
                w = wpool.tile([128, 3 * H], F32, tag=f"whh{k}")
                nc.sync.dma_start(out=w[:], in_=whh_in[128 * k:128 * (k + 1), :])
                whh_sb.append(w)
            bp_sb = wpool.tile([1, VS], F32, tag="bp")
            nc.sync.dma_start(out=bp_sb[:], in_=bproj_in[:, :])
            bnh_sb = wpool.tile([1, H], F32, tag="bnh")
            nc.sync.dma_start(out=bnh_sb[:], in_=bnh_in[:, :])
            ident_sb = wpool.tile([B, B], F32, tag="ident")
            nc.sync.dma_start(out=ident_sb[:], in_=ident_in[:, :])
            ident64_sb = wpool.tile([2 * B, B], F32, tag="ident64")
            nc.sync.dma_start(out=ident64_sb[:], in_=ident64_in[:, :])
            ones_sb = wpool.tile([1, B], F32, tag="ones")
            nc.sync.dma_start(out=ones_sb[:], in_=ones_in[:, :])
            pbase_sb = wpool.tile([128, 1], F32, tag="pbase")
            nc.sync.dma_start(out=pbase_sb[:], in_=pbase_in[:, :])

            # ---------- decode state ----------
            toks_sb = state.tile([B, T], F32, tag="toks")
            eos_f = state.tile([B, 1], F32, tag="eos")
            nc.vector.memset(eos_f[:], float(EOS))

            h_cur = sb.tile([2 * B, 256], F32, tag="h")
            nc.sync.dma_start(out=h_cur[:], in_=h0_in[:, :])
            hT_cur = sb.tile([128, 4, B], F32, tag="hT")
            nc.sync.dma_start(
                out=hT_cur[:],
                in_=h0T_in.ap().rearrange("(k p) b -> p k b", p=128),
            )
            # fp16 hi/lo split of hT (exact: hi+lo == hT to ~2^-24)
            hTh_cur = sb.tile([128, 4, B], F16, tag="hTh")
            nc.vector.tensor_copy(hTh_cur[:], hT_cur[:])
            hTl32 = sb.tile([128, 4, B], F32, tag="hTl32")
            nc.vector.tensor_copy(hTl32[:], hTh_cur[:])
            nc.vector.tensor_tensor(hTl32[:], hT_cur[:], hTl32[:], op=OP.subtract)
            hTl_cur = sb.tile([128, 4, B], F16, tag="hTl")
            nc.vector.tensor_copy(hTl_cur[:], hTl32[:])

            tok_f = sb.tile([B, 1], F32, tag="tok")
            nc.vector.memset(tok_f[:], float(SOS))
            done_u = sb.tile([B, 1], U32, tag="done")
            nc.vector.memset(done_u[:], 0)

            def issue_gates(hT):
                """recurrent gate pre-activations from hT (overlap window).
                [64, 256] layout: partition 32*q + b holds h-range 256q+j,
                so the serial DVE/ACT chain runs at 64 lanes instead of 32."""
                g_r = ps_gate.tile([2 * B, 256], F32, tag="g_r")
                g_z = ps_gate.tile([2 * B, 256], F32, tag="g_z")
                g_hn = ps_gate.tile([2 * B, 256], F32, tag="g_hn")
                for gi, gt in ((0, g_r), (1, g_z)):
                    for q in range(2):
                        for k in range(4):
                            nc.tensor.matmul(
                                gt[B * q:B * (q + 1), :], hT[:, k, :],
                                whh_sb[k][:, gi * H + 256 * q:
                                          gi * H + 256 * q + 256],
                                start=(k == 0), stop=(k == 3),
                                tile_position=(0, B * q))
                for q in range(2):
                    nc.tensor.matmul(g_hn[B * q:B * (q + 1), :], ones_sb[:1, :],
                                     bnh_sb[:1, 256 * q:256 * q + 256],
                                     start=True, stop=False,
                                     tile_position=(0, B * q))
                    for k in range(4):
                        nc.tensor.matmul(
                            g_hn[B * q:B * (q + 1), :], hT[:, k, :],
                            whh_sb[k][:, 2 * H + 256 * q:2 * H + 256 * q + 256],
                            start=False, stop=(k == 3),
                            tile_position=(0, B * q))
                return g_r, g_z, g_hn

            def issue_proj_bias():
                """bias-init the two projection half psums (overlap window)"""
                pjs = [ps_proj.tile([128, 512], F32, tag="pj0", name="pj0"),
                       ps_proj.tile([128, 512], F32, tag="pj1", name="pj1")]
                for tt in range(2):
                    for g in range(4):
                        nc.tensor.matmul(
                            pjs[tt][32 * g:32 * (g + 1), :],
                            ones_sb[:1, :],
                            bp_sb[:1, g * 1024 + tt * 512:g * 1024 + tt * 512 + 512],
                            start=True, stop=False,
                            tile_position=(0, 32 * g),
                        )
                return pjs

            def issue_dummies(hT):
                """keep the PE HAM clock-gate warm through the exchange wait"""
                scratch = ps_dummy.tile([128, 512], F32, tag="scr")
                for _ in range(N_DUMMY):
                    nc.tensor.matmul(scratch[0:B, :], hT[:, 0, :], whh_sb[0][:, 0:512],
                                     start=True, stop=True)

            def heartbeat(x_ap, n_cols):
                """a tiny matmul gated on x: fires when x is produced, keeping
                the PE HAM clock-gate warm through the token-dependent phase.
                x is the STATIONARY operand (LDWEIGHTS waits on it) and the
                moving side is a 32-col identity, so each beat costs ~0.2us."""
                scratch = ps_dummy.tile([128, 512], F32, tag="scr")
                nc.tensor.matmul(scratch[0:n_cols, 0:B], x_ap, ident_sb[:, :],
                                 start=True, stop=True)

            g_r, g_z, g_hn = issue_gates(hT_cur)
            pjs = issue_proj_bias()
            issue_dummies(hT_cur)

            for t in range(T):
                # ---- gate input rows: gx = gtab[tok] (= x@W_ih.T + biases) ----
                tok_u = sb.tile([B, 1], U32, tag="tok_u")
                nc.vector.tensor_copy(tok_u[:], tok_f[:])
                gx64 = sb.tile([2 * B, 3 * 256], F32, tag="gx64")
                for q in range(2):
                    nc.gpsimd.indirect_dma_start(
                        out=gx64[B * q:B * (q + 1), :],
                        out_offset=None,
                        in_=gtabq_in[q][:, :],
                        in_offset=bass.IndirectOffsetOnAxis(ap=tok_u[:, :1], axis=0),
                    )
                heartbeat(tok_f[:], 1)
                heartbeat(gx64[0:B, 0:128], 128)

                # ---- gates: sigmoid via tanh (4-ULP table) ----
                # r-chain on DVE, z-chain on GpSimd (parallel engines)
                rp = sb.tile([2 * B, 256], F32, tag="rp")
                nc.vector.tensor_tensor(rp[:], g_r[:], gx64[:, 0:256], op=OP.add)
                heartbeat(rp[0:B, 0:128], 128)
                zp = sb.tile([2 * B, 256], F32, tag="zp")
                nc.vector.tensor_tensor(zp[:], g_z[:], gx64[:, 256:512], op=OP.add)
                rt = sb.tile([2 * B, 256], F32, tag="rt")
                nc.scalar.activation(rt[:], rp[:], AF.Tanh, scale=0.5)
                zt = sb.tile([2 * B, 256], F32, tag="zt")
                nc.scalar.activation(zt[:], zp[:], AF.Tanh, scale=0.5)
                heartbeat(zt[0:B, 0:128], 128)
                omz = sb.tile([2 * B, 256], F32, tag="omz")   # 1 - z
                nc.vector.tensor_scalar(omz[:], zt[:], -0.5, 0.5, op0=OP.mult,
                                        op1=OP.add)
                zs = sb.tile([2 * B, 256], F32, tag="zs")     # z
                nc.vector.tensor_scalar(zs[:], zt[:], 0.5, 0.5, op0=OP.mult,
                                        op1=OP.add)
                zh = sb.tile([2 * B, 256], F32, tag="zh")     # z*h
                nc.vector.tensor_tensor(zh[:], zs[:], h_cur[:], op=OP.mult)

                # r*ghn + gxn, with r = 0.5*(rt+1): tmp = 0.5*((rt+1)*ghn) + gxn
                tmp1 = sb.tile([2 * B, 256], F32, tag="tmp1")
                nc.vector.scalar_tensor_tensor(tmp1[:], rt[:], 1.0, g_hn[:],
                                               op0=OP.add, op1=OP.mult)
                tmp = sb.tile([2 * B, 256], F32, tag="tmp")
                nc.vector.scalar_tensor_tensor(tmp[:], tmp1[:], 0.5,
                                               gx64[:, 512:768],
                                               op0=OP.mult, op1=OP.add)
                heartbeat(tmp[0:B, 0:128], 128)
                n_sb = sb.tile([2 * B, 256], F32, tag="n")
                nc.scalar.activation(n_sb[:], tmp[:], AF.Tanh)
                heartbeat(n_sb[0:B, 0:128], 128)
                h_new = sb.tile([2 * B, 256], F32, tag="h")
                nc.vector.tensor_tensor(h_new[:], omz[:], n_sb[:], op=OP.mult)
                nc.vector.tensor_tensor(h_new[:], h_new[:], zh[:], op=OP.add)

                # ---- hT (PE transpose; two psum tiles so copies pipeline) ----
                hT_psA = ps_tp.tile([128, 2, B], F32, tag="tpA")
                hT_psB = ps_tp.tile([128, 2, B], F32, tag="tpB")
                for kk in range(4):
                    q, hh = kk // 2, kk % 2
                    dst = hT_psA if kk < 2 else hT_psB
                    nc.tensor.transpose(
                        dst[:, kk % 2, :],
                        h_new[B * q:B * (q + 1), 128 * hh:128 * (hh + 1)],
                        ident64_sb[B * q:B * (q + 1), :],
                    )
                hT_new = sb.tile([128, 4, B], F32, tag="hT")
                nc.vector.tensor_copy(hT_new[:, 0:2, :], hT_psA[:])
                nc.vector.tensor_copy(hT_new[:, 2:4, :], hT_psB[:])

                # ---- fp16 hi/lo split of hT ----
                hTh = sb.tile([128, 4, B], F16, tag="hTh")
                nc.vector.tensor_copy(hTh[:], hT_new[:])
                hTl32n = sb.tile([128, 4, B], F32, tag="hTl32")
                nc.vector.tensor_copy(hTl32n[:], hTh[:])
                nc.vector.tensor_tensor(hTl32n[:], hT_new[:], hTl32n[:],
                                        op=OP.subtract)
                hTl = sb.tile([128, 4, B], F16, tag="hTl")
                nc.vector.tensor_copy(hTl[:], hTl32n[:])

                # ---- projection: exact fp16 3-pass (hi@hi + hi@lo + lo@hi),
                # half tt=0 fully first so its argmax overlaps tt=1's matmuls
                for tt in range(2):
                    for pi, (hTp, wp) in enumerate(((hTh, wph_sb), (hTh, wpl_sb),
                                                    (hTl, wph_sb))):
                        for k in range(4):
                            last = (pi == 2 and k == 3)
                            for g in range(4):
                                nc.tensor.matmul(
                                    pjs[tt][32 * g:32 * (g + 1), :],
                                    hTp[:, k, :],
                                    wp[k][:, g * 1024 + tt * 512:
                                          g * 1024 + tt * 512 + 512],
                                    start=False, stop=last,
                                    tile_position=(0, 32 * g),
                                )
                    if tt == 0:
                        mxa = sb.tile([128, 8], F32, tag="mxa")
                        mia = sb.tile([128, 8], U32, tag="mia")
                        nc.vector.max(out=mxa[:], in_=pjs[0][:, :])
                        nc.vector.max_index(mia[:], mxa[:], pjs[0][:, :])

                # ---- half-1 argmax + merge (strictly-greater keeps half 0,
                # i.e. the lower vocab index, matching jnp.argmax ties) ----
                mxb = sb.tile([128, 8], F32, tag="mxb")
                mib = sb.tile([128, 8], U32, tag="mib")
                nc.vector.max(out=mxb[:], in_=pjs[1][:, :])
                nc.vector.max_index(mib[:], mxb[:], pjs[1][:, :])
                ia = sb.tile([128, 1], F32, tag="ia")
                nc.vector.tensor_copy(ia[:], mia[:, 0:1])
                ib = sb.tile([128, 1], F32, tag="ib")
                nc.vector.tensor_copy(ib[:], mib[:, 0:1])
                nc.vector.tensor_scalar(ib[:], ib[:], 512.0, None, op0=OP.add)
                mcmp = sb.tile([128, 1], U32, tag="mcmp")
                nc.vector.tensor_tensor(mcmp[:], mxb[:, 0:1], mxa[:, 0:1], op=OP.is_gt)
                cand = sb.tile([128, 2], F32, tag="cand")
                nc.vector.tensor_copy(cand[:, 0:1], mxa[:, 0:1])
                nc.vector.copy_predicated(cand[:, 0:1], mcmp[:], mxb[:, 0:1])
                nc.vector.copy_predicated(ia[:], mcmp[:], ib[:])
                nc.vector.tensor_tensor(cand[:, 1:2], ia[:], pbase_sb[:], op=OP.add)

                # ---- exchange across cores (cand DMA on the idle scalar queue) --
                nc.scalar.dma_start(out=cc_ins[t][:, :], in_=cand[:])
                nc.gpsimd.collective_compute(
                    "AllGather",
                    mybir.AluOpType.bypass,
                    replica_groups=[list(range(N_CORES))],
                    ins=[cc_ins[t].ap().opt()],
                    outs=[cc_outs[t].ap().opt()],
                )

                # ---- overlap window: issue t+1 PE work while AllGather runs ----
                g_r, g_z, g_hn = issue_gates(hT_new)
                pjs = issue_proj_bias()
                issue_dummies(hT_new)

                # ---- readback (single strided DMA on the idle scalar queue) ----
                gath = sb.tile([B, 32, 2], F32, tag="gath")
                nc.scalar.dma_start(
                    out=gath[:, 0:16, :],
                    in_=cc_outs[t][0:512, :].rearrange("(r g b) c -> b (r g) c", r=4, g=4),
                )
                nc.sync.dma_start(
                    out=gath[:, 16:32, :],
                    in_=cc_outs[t][512:1024, :].rearrange("(r g b) c -> b (r g) c", r=4, g=4),
                )

                heartbeat(gath[:, 0:8, 0:2], 16)

                # ---- global winner: max value, then reconstruct its index by
                # value-match (exact fp32 value ties across cores are measure-zero)
                wmax = sb.tile([B, 8], F32, tag="wmax")
                nc.vector.max(out=wmax[:], in_=gath[:, :, 0:1])
                weq = sb.tile([B, 32], F32, tag="weq")
                nc.vector.tensor_scalar(weq[:], gath[:, :, 0:1], wmax[:, 0:1],
                                        None, op0=OP.is_equal)
                nc.vector.tensor_tensor(weq[:], weq[:], gath[:, :, 1:2],
                                        op=OP.mult)
                y_new_t = sb.tile([B, 1], F32, tag="ynew")
                nc.vector.tensor_reduce(y_new_t[:], weq[:], axis=mybir.AxisListType.X,
                                        op=OP.max)
                y_new = y_new_t[:]

                # ---- token bookkeeping (tok_f first: the next gather needs it) ----
                tok_f = sb.tile([B, 1], F32, tag="tok")
                nc.vector.tensor_copy(tok_f[:], y_new)
                nc.vector.copy_predicated(tok_f[:], done_u[:], eos_f[:])
                eq_u = sb.tile([B, 1], U32, tag="eq")
                nc.vector.tensor_tensor(eq_u[:], y_new, eos_f[:], op=OP.is_equal)
                done_new = sb.tile([B, 1], U32, tag="done")
                nc.vector.tensor_tensor(done_new[:], done_u[:], eq_u[:], op=OP.bitwise_or)
                nc.vector.tensor_copy(toks_sb[:, t:t + 1], tok_f[:])
                done_u = done_new
                h_cur = h_new
                hT_cur = hT_new
                hTh_cur = hTh
                hTl_cur = hTl

            toks_i = state.tile([B, T], I32, tag="toks_i")
            nc.vector.tensor_copy(toks_i[:], toks_sb[:])
            nc.sync.dma_start(out=toks_out[:, :], in_=toks_i[:])

    nc.compile()
    return nc


_NC_CACHE = {}
TRACE = False
LAST_EXEC_NS = None


def kernel(hidden, emb, W_ih, W_hh, b_ih, b_hh, W_proj, b_proj, max_len, **_):
    from concourse.bass_utils import run_bass_kernel_spmd

    T = int(max_len)
    hidden = np.asarray(hidden, dtype=np.float32)
    emb = np.asarray(emb, dtype=np.float32)
    W_ih = np.asarray(W_ih, dtype=np.float32)
    W_hh = np.asarray(W_hh, dtype=np.float32)
    b_ih = np.asarray(b_ih, dtype=np.float32)
    b_hh = np.asarray(b_hh, dtype=np.float32)
    W_proj = np.asarray(W_proj, dtype=np.float32)
    b_proj = np.asarray(b_proj, dtype=np.float32)

    # input-side gate table: gtab[v] = emb[v] @ W_ih.T (+ r,z biases / x-side n bias)
    gtab = emb @ np.ascontiguousarray(W_ih.T)
    gtab[:, 0:2 * H] += (b_ih + b_hh)[None, 0:2 * H]
    gtab[:, 2 * H:3 * H] += b_ih[None, 2 * H:3 * H]
    gtab = np.ascontiguousarray(gtab, dtype=np.float32)

    # pad vocab so every core owns exactly VS rows; padded logits = -1e30
    Wp = np.zeros((VPAD, H), dtype=np.float32)
    Wp[:V] = W_proj
    bp = np.full((VPAD,), NEG, dtype=np.float32)
    bp[:V] = b_proj

    whhT = np.ascontiguousarray(W_hh.T)
    b_nh = np.ascontiguousarray(b_hh[None, 2 * H:3 * H])
    h0 = np.ascontiguousarray(hidden[0])
    h0T = np.ascontiguousarray(h0.T)
    h0q = np.ascontiguousarray(
        h0.reshape(B, 2, 256).transpose(1, 0, 2).reshape(2 * B, 256))
    gtabq = [np.ascontiguousarray(
        gtab.reshape(V, 3, 2, 256)[:, :, q, :].reshape(V, 768))
        for q in range(2)]
    ident = np.eye(B, dtype=np.float32)
    ident64 = np.concatenate([np.eye(B, dtype=np.float32)] * 2, axis=0)
    ones = np.ones((1, B), dtype=np.float32)

    if T not in _NC_CACHE:
        _NC_CACHE[T] = _build(T)
    nc = _NC_CACHE[T]

    in_maps = []
    for c in range(N_CORES):
        pbase = (c * VS + (np.arange(128) // 32) * 1024).astype(np.float32)[:, None]
        wT = np.ascontiguousarray(Wp[c * VS:(c + 1) * VS].T)  # [H, VS] fp32
        wph = wT.astype(np.float16)
        wpl = (wT - wph.astype(np.float32)).astype(np.float16)
        in_maps.append({
            "wph": wph,
            "wpl": wpl,
            "bproj": np.ascontiguousarray(bp[None, c * VS:(c + 1) * VS]),
            "whhT": whhT,
            "b_nh": b_nh,
            "gtabq0": gtabq[0], "gtabq1": gtabq[1],
            "h0": h0q, "h0T": h0T,
            "ident": ident, "ident64": ident64, "ones": ones,
            "pbase": pbase,
        })

    global LAST_EXEC_NS
    res = run_bass_kernel_spmd(nc, in_maps, core_ids=list(range(N_CORES)), trace=TRACE)
    LAST_EXEC_NS = res.exec_time_ns
    toks = res.results[0]["toks"]
    return np.ascontiguousarray(toks.T.astype(np.int32))
